# revision 43
# baseline (speedup 1.0000x reference)
"""Expert-parallel MoE SwiGLU kernel for one TRN2 chip (8 NeuronCores).

Problem: out[n] = sum_k w[n,k] * FFN_{idx[n,k]}(x[n]) with E=8 experts,
top-2 routing, H=1024, I=4096, N=2048 tokens.

Strategy: one expert per core. Tokens are routed (gathered) per expert on
the host, each core runs the three bf16 matmuls of its expert's SwiGLU FFN
(silu(x@w1) * (x@w3)) @ w2 over its token batch entirely transposed
(tokens along the PE moving/free dim), and the host scatter-adds the
returned per-expert outputs with the routing weights. Expert token counts
above the per-core capacity (PE moving-dim chunk of 512) spill to a small
host-side f32 pass so the device runs a single full-width chunk.

Schedule notes (from NTFF profile analysis): the kernel is tensor-bound
(768 N=512 bf16 matmuls ~= 166us at 2.4GHz; fp8 is ruled out by the 2e-2
relative-error budget). The startup-critical inputs (x 1MB on the sync
DMA ring, the first w13 tile 0.5MB on the scalar ring) land ~14-16us
after kernel start; warmup matmuls ramp the HAM clock and keep the PE
busy until then, after which the matmul stream runs dense (216ns/matmul)
to the end. The w13 weight stream arrives as 1MB pair tiles alternating
between the scalar and gpsimd rings (larger DMA lines run faster; fewer
triggers and semaphores shorten the teardown), w2 likewise in Phase B.
The final output chunk is split into two pipelined copy+store pieces on
both hardware DGE rings to shorten the kernel tail.
"""

import sys

for _p in ("/opt/trn_rl_repo", "/opt/pypackages"):
    if _p not in sys.path:
        sys.path.insert(0, _p)

import numpy as np
import ml_dtypes

import concourse.tile as tile
from concourse import bacc, mybir
from concourse.bass_utils import run_bass_kernel_spmd

P = 128
H = 1024
I = 4096
KH = H // P    # 8 contraction subtiles for the first matmuls
KH2 = KH // 2  # kh subtiles per x half-tile
II = I // P    # 32 intermediate subtiles / contraction subtiles for w2
CAP = 512      # per-core token capacity (single PE moving chunk)
# PE warmup matmuls: ramp the HAM clock AND keep the tensor engine busy
# until the startup-critical DMAs are fully resident (~6us after the
# first warmup matmul). Overshoot costs 109ns/matmul; undershoot risks a
# HAM re-throttle that halves the early real matmul rate.
N_WARM = 46
W_COLS = 256   # warmup matmul moving width (finer tail granularity)

BF16 = mybir.dt.bfloat16
F32 = mybir.dt.float32


def _build(C):
    """One-expert SwiGLU FFN over C tokens (C <= 512), transposed layout.

    DRAM inputs (per core):
      xg   [P, KH, C]       bf16  x^T: [hp, kh, c] = x[tok c, kh*P+hp]
      w13t [II/2, P, 2, 2, KH, P] bf16, pair-major ii blocks:
           [j, hp, a, 0, kh, m] = w1[kh*P+hp, (2j+a)*P+m]
           [j, hp, a, 1, kh, m] = w3[kh*P+hp, (2j+a)*P+m]
      w2t  [KH/2, P, 2, II, P] bf16, pair-major hh blocks:
           [q, ip, b, ik, m] = w2[ik*P+ip, (2q+b)*P+m]
    Output:
      yt   [KH/2, P, 2, C]  f32   y^T, pair-major output subtiles
    """
    assert C <= 512
    nc = bacc.Bacc("TRN2", target_bir_lowering=False, debug=False)
    xg = nc.dram_tensor("xg", [P, KH, C], BF16, kind="ExternalInput")
    w13t = nc.dram_tensor(
        "w13t", [II // 2, P, 2, 2, KH, P], BF16, kind="ExternalInput"
    )
    w2t = nc.dram_tensor(
        "w2t", [KH // 2, P, 2, II, P], BF16, kind="ExternalInput"
    )
    yt = nc.dram_tensor("yt", [KH // 2, P, 2, C], F32, kind="ExternalOutput")

    with tile.TileContext(nc) as tc:
        with (
            tc.tile_pool(name="xp", bufs=1) as xp,
            tc.tile_pool(name="pp", bufs=1) as pp,
            tc.tile_pool(name="wp", bufs=10) as wp,
            tc.tile_pool(name="w2p", bufs=2) as w2p,
            tc.tile_pool(name="gp", bufs=4) as gp,
            tc.tile_pool(name="yp", bufs=3) as yp,
            tc.tile_pool(name="warm", bufs=1) as warm,
            tc.tile_pool(name="psA", bufs=3, space="PSUM") as psA,
            tc.tile_pool(name="psB", bufs=2, space="PSUM") as psB,
        ):
            # Startup-critical loads. DMA ring throughput scales with the
            # per-partition line size (2KB lines ~80GB/s, 4KB ~150GB/s, 8KB
            # ~200+GB/s), so keep transfers WHOLE: x (8KB lines) rides the
            # sync ring (up ~8.7us), w13[0] (4KB lines) the scalar ring
            # (~9.9us, after its ACT table load). Both land ~14us.
            # high_priority pins these triggers at the head of their engine
            # queues — in particular ahead of the two ACT_TABLE_LOADs
            # (2.56us) the framework schedules on the scalar queue, which
            # otherwise delay the w13[0] descriptor submission.
            with tc.high_priority():
                xsb = xp.tile([P, KH, C], BF16)
                nc.sync.dma_start(xsb[:], xg[:])
                # ii=0 alone (512KB): keeps the scalar-ring transfer short
                # so even with ring-start jitter the warmup->data gap stays
                # under the ~3.4us HAM idle window (a 1MB pair here was
                # observed to blow past it and re-throttle the PE clock).
                w13sb0 = wp.tile([P, 2, KH, P], BF16, tag="w13", bufs=4)
                nc.scalar.dma_start(w13sb0[:], w13t[0][:, 0])

            def xh(kh):
                return xsb[:, kh, :]

            # PE warmup: ramp the tensor engine to high-activity clock while
            # the input DMAs are in flight. Reads a zeroed tile, result is
            # never consumed.
            wtile = warm.tile([P, W_COLS], BF16)
            nc.vector.memset(wtile[:], 0.0)
            # Shares the Phase B psum pool (tag "py"): warmup is long done
            # before Phase B allocates its first chain psum.
            wps = psB.tile([P, W_COLS], F32, tag="py")
            for i in range(N_WARM):
                nc.tensor.matmul(
                    wps, wtile[:, :P], wtile[:], start=(i == 0),
                    stop=(i == N_WARM - 1),
                )

            psb = pp.tile([P, II, C], BF16)

            # Phase A: h1 = silu(x@w1), h3 = x@w3, p = h1*h3 (all transposed)
            # w13 stream: ii=1..3 as single tiles on the gpsimd ring (just-in-
            # time at startup), ii>=4 as pair tiles (one DMA per two ii —
            # fewer triggers and semaphores). The scalar ring, free once the
            # startup x chunks land, carries most pairs; gpsimd the rest.
            wpair = None
            for ii in range(II):
                j, a = divmod(ii, 2)
                if ii == 0:
                    wsel = lambda half, kh: w13sb0[:, half, kh, :]
                elif ii in (1, 2, 3):
                    # Just-in-time singles: ii=1,2 on the gpsimd software-DGE
                    # ring (its first data lands ~15us, one tile per ~3.4us);
                    # ii=3 rides the scalar ring right behind w13[0] — the
                    # gpsimd ring delivers it ~1us too late.
                    wsb = wp.tile([P, 2, KH, P], BF16, tag="w13", bufs=4)
                    eng = nc.scalar if ii == 3 else nc.gpsimd
                    eng.dma_start(wsb[:], w13t[j][:, a])
                    wsel = lambda half, kh, t=wsb: t[:, half, kh, :]
                elif a == 0:
                    wpair = wp.tile([P, 2, 2, KH, P], BF16, tag="w13p", bufs=5)
                    # j=2,3 on scalar (gpsimd is busy with the JIT singles
                    # until ~22us); afterwards alternate rings.
                    eng = nc.scalar if (j < 4 or j % 2 == 1) else nc.gpsimd
                    eng.dma_start(wpair[:], w13t[j])
                    wsel = lambda half, kh, t=wpair: t[:, 0, half, kh, :]
                else:
                    wsel = lambda half, kh, t=wpair: t[:, 1, half, kh, :]
                pg = psA.tile([P, C], F32, tag="pg")
                pu = psA.tile([P, C], F32, tag="pu")
                for kh in range(KH):
                    nc.tensor.matmul(
                        pg,
                        wsel(0, kh),
                        xh(kh),
                        start=(kh == 0),
                        stop=(kh == KH - 1),
                    )
                for kh in range(KH):
                    nc.tensor.matmul(
                        pu,
                        wsel(1, kh),
                        xh(kh),
                        start=(kh == 0),
                        stop=(kh == KH - 1),
                    )
                gs = gp.tile([P, C], BF16, tag="g")
                nc.scalar.activation(gs, pg, mybir.ActivationFunctionType.Silu)
                nc.vector.tensor_tensor(
                    psb[:, ii, :], gs, pu, mybir.AluOpType.mult
                )

            # Phase B: y = p @ w2 (transposed: yT = w2T-contraction over I).
            # w2 arrives as 1MB pair tiles (hh 2q, 2q+1 together) and y goes
            # out as pair stores — fewer DMA triggers and semaphores (the
            # teardown zeroes every semaphore inside the measured window).
            # The last hh is split column-wise so its first half's copy+DMA
            # overlaps the second half's matmuls (shorter kernel tail).
            w2sb = yd = None
            for hh in range(KH):
                q, b = divmod(hh, 2)
                if b == 0:
                    w2sb = w2p.tile([P, 2, II, P], BF16, tag="w2")
                    # first pair on scalar (its w13 stream drains first at
                    # the A->B transition; gpsimd is still backlogged),
                    # then alternate.
                    eng = nc.scalar if q % 2 == 0 else nc.gpsimd
                    eng.dma_start(w2sb[:], w2t[q])
                    yd = yp.tile([P, 2, C], F32, tag="y2")
                halves = [(0, C)] if hh < KH - 1 else [
                    (0, C // 2), (C // 2, C - C // 2),
                ]
                for hi, (c0, cc) in enumerate(halves):
                    py = psB.tile([P, cc], F32, tag="py")
                    for ik in range(II):
                        nc.tensor.matmul(
                            py,
                            w2sb[:, b, ik, :],
                            psb[:, ik, c0 : c0 + cc],
                            start=(ik == 0),
                            stop=(ik == II - 1),
                        )
                    # DVE copies keep the COPY activation table off the
                    # scalar queue (its ACT_TABLE_LOAD would delay the scalar
                    # DMA ring's startup-critical triggers by ~1.3us).
                    if hh < KH - 1 or hi == 0:
                        nc.vector.tensor_copy(yd[:, b, c0 : c0 + cc], py)
                        if b == 1 and hh < KH - 1:
                            nc.sync.dma_start(yt[q], yd[:])
                        elif hh == KH - 1:
                            # penultimate store: hh=6 whole + hh=7 first
                            # half, one trigger
                            nc.sync.dma_start(
                                yt[q, :, 0, :], yd[:, 0, :]
                            )
                            nc.sync.dma_start(
                                yt[q, :, 1, c0 : c0 + cc],
                                yd[:, 1, c0 : c0 + cc],
                            )
                    else:
                        # Final piece on the other hardware DGE ring,
                        # pipelined behind the first half's store.
                        yb = yp.tile([P, cc], F32, tag="y")
                        nc.vector.tensor_copy(yb, py)
                        nc.scalar.dma_start(
                            yt[q, :, 1, c0 : c0 + cc], yb[:]
                        )

    nc.compile()
    return nc


_PROGRAM_CACHE = {}


def _host_swiglu(x, w1e, w2e, w3e):
    g = x @ w1e
    u = x @ w3e
    g = g / (1.0 + np.exp(-g))
    return (g * u) @ w2e


def kernel(x, expert_indices, expert_weights, w1, w2, w3):
    x = np.asarray(x, dtype=np.float32)
    idx = np.asarray(expert_indices)
    wts = np.asarray(expert_weights, dtype=np.float32)
    w1 = np.asarray(w1, dtype=np.float32)
    w2 = np.asarray(w2, dtype=np.float32)
    w3 = np.asarray(w3, dtype=np.float32)
    N = x.shape[0]
    E = w1.shape[0]
    bf16 = ml_dtypes.bfloat16

    # host-side routing: token list (with multiplicity) per expert; tokens
    # beyond CAP spill to the host f32 path (tiny tail, keeps device at one
    # full-width PE chunk)
    toks, tokw, spill_toks, spill_w = [], [], [], []
    for e in range(E):
        rows, cols = np.nonzero(idx == e)
        w_e = wts[rows, cols]
        toks.append(rows[:CAP])
        tokw.append(w_e[:CAP])
        spill_toks.append(rows[CAP:])
        spill_w.append(w_e[CAP:])
    C = max(16, max(len(t) for t in toks))
    C = ((C + 15) // 16) * 16

    if C not in _PROGRAM_CACHE:
        _PROGRAM_CACHE[C] = _build(C)
    nc = _PROGRAM_CACHE[C]

    in_maps = []
    for e in range(E):
        xt = np.zeros((C, H), dtype=np.float32)
        if len(toks[e]):
            xt[: len(toks[e])] = x[toks[e]]
        # [C, H] -> [hp, kh, c]
        xge = xt.T.reshape(KH, P, C).transpose(1, 0, 2)
        # w1/w3 [H, I] -> [ii, hp, {w1,w3}, kh, m] -> pair-major
        # [ii/2, hp, ii%2, {w1,w3}, kh, m]
        w13 = np.stack(
            [
                w1[e].reshape(KH, P, II, P).transpose(2, 1, 0, 3),
                w3[e].reshape(KH, P, II, P).transpose(2, 1, 0, 3),
            ],
            axis=2,
        )  # [II, P, 2, KH, P]
        w13 = w13.reshape(II // 2, 2, P, 2, KH, P).swapaxes(1, 2)
        in_maps.append(
            {
                "xg": np.ascontiguousarray(xge.astype(bf16)),
                "w13t": np.ascontiguousarray(w13.astype(bf16)),
                "w2t": np.ascontiguousarray(
                    w2[e].reshape(II, P, KH, P).transpose(2, 1, 0, 3)
                    .reshape(KH // 2, 2, P, II, P).swapaxes(1, 2).astype(bf16)
                ),
            }
        )

    res = run_bass_kernel_spmd(nc, in_maps, core_ids=list(range(E)))

    out = np.zeros((N, H), dtype=np.float32)
    for e in range(E):
        cnt = len(toks[e])
        if cnt:
            y = (
                res.results[e]["yt"]
                .reshape(KH // 2, P, 2, C)
                .swapaxes(1, 2)
                .reshape(H, C)
                .T[:cnt]
            )
            np.add.at(out, toks[e], y * tokw[e][:, None])
        if len(spill_toks[e]):
            ys = _host_swiglu(x[spill_toks[e]], w1[e], w2[e], w3[e])
            np.add.at(out, spill_toks[e], ys * spill_w[e][:, None])
    return out


# revision 44
# speedup vs baseline: 1.1870x; 1.1870x over previous
"""Expert-parallel MoE SwiGLU kernel for one TRN2 chip (8 NeuronCores).

Problem: out[n] = sum_k w[n,k] * FFN_{idx[n,k]}(x[n]) with E=8 experts,
top-2 routing, H=1024, I=4096, N=2048 tokens.

Strategy: one expert per core. Tokens are routed (gathered) per expert on
the host, each core runs the three bf16 matmuls of its expert's SwiGLU FFN
(silu(x@w1) * (x@w3)) @ w2 over its token batch entirely transposed
(tokens along the PE moving/free dim), and the host scatter-adds the
returned per-expert outputs with the routing weights. Expert token counts
above the per-core capacity (PE moving-dim chunk of 512) spill to a small
host-side f32 pass so the device runs a single full-width chunk.

Schedule notes (from NTFF profile analysis): the kernel is tensor-bound
(768 N=512 bf16 matmuls ~= 166us at 2.4GHz; fp8 is ruled out by the 2e-2
relative-error budget). The startup-critical inputs (x 1MB on the sync
DMA ring, the first w13 tile 0.5MB on the scalar ring) land ~14-16us
after kernel start; warmup matmuls ramp the HAM clock and keep the PE
busy until then, after which the matmul stream runs dense (216ns/matmul)
to the end. The w13 weight stream arrives as 1MB pair tiles alternating
between the scalar and gpsimd rings (larger DMA lines run faster; fewer
triggers and semaphores shorten the teardown), w2 likewise in Phase B.
The final output chunk is split into two pipelined copy+store pieces on
both hardware DGE rings to shorten the kernel tail.
"""

import sys

for _p in ("/opt/trn_rl_repo", "/opt/pypackages"):
    if _p not in sys.path:
        sys.path.insert(0, _p)

import numpy as np
import ml_dtypes

import concourse.tile as tile
from concourse import bacc, mybir
from concourse.bass_utils import run_bass_kernel_spmd

P = 128
H = 1024
I = 4096
KH = H // P    # 8 contraction subtiles for the first matmuls
KH2 = KH // 2  # kh subtiles per x half-tile
II = I // P    # 32 intermediate subtiles / contraction subtiles for w2
CAP = 512      # per-core token capacity (single PE moving chunk)
# PE warmup matmuls: ramp the HAM clock AND keep the tensor engine busy
# until the startup-critical DMAs are fully resident (~6us after the
# first warmup matmul). Overshoot costs 109ns/matmul; undershoot risks a
# HAM re-throttle that halves the early real matmul rate.
N_WARM = 52
W_COLS = 256   # warmup matmul moving width (finer tail granularity)

BF16 = mybir.dt.bfloat16
F32 = mybir.dt.float32


def _build(C):
    """One-expert SwiGLU FFN over C tokens (C <= 512), transposed layout.

    DRAM inputs (per core):
      xg   [P, KH, C]       bf16  x^T: [hp, kh, c] = x[tok c, kh*P+hp]
      w13t [II/2, P, 2, 2, KH, P] bf16, pair-major ii blocks:
           [j, hp, a, 0, kh, m] = w1[kh*P+hp, (2j+a)*P+m]
           [j, hp, a, 1, kh, m] = w3[kh*P+hp, (2j+a)*P+m]
      w2t  [KH/2, P, 2, II, P] bf16, pair-major hh blocks:
           [q, ip, b, ik, m] = w2[ik*P+ip, (2q+b)*P+m]
    Output:
      yt   [KH/2, P, 2, C]  f32   y^T, pair-major output subtiles
    """
    assert C <= 512
    nc = bacc.Bacc("TRN2", target_bir_lowering=False, debug=False)
    xg = nc.dram_tensor("xg", [P, KH, C], BF16, kind="ExternalInput")
    w13t = nc.dram_tensor(
        "w13t", [II // 2, P, 2, 2, KH, P], BF16, kind="ExternalInput"
    )
    w2t = nc.dram_tensor(
        "w2t", [KH // 2, P, 2, II, P], BF16, kind="ExternalInput"
    )
    yt = nc.dram_tensor("yt", [KH // 2, P, 2, C], F32, kind="ExternalOutput")

    with tile.TileContext(nc) as tc:
        with (
            tc.tile_pool(name="xp", bufs=1) as xp,
            tc.tile_pool(name="pp", bufs=1) as pp,
            tc.tile_pool(name="wp", bufs=10) as wp,
            tc.tile_pool(name="w2p", bufs=2) as w2p,
            tc.tile_pool(name="gp", bufs=4) as gp,
            tc.tile_pool(name="yp", bufs=3) as yp,
            tc.tile_pool(name="warm", bufs=1) as warm,
            tc.tile_pool(name="psA", bufs=3, space="PSUM") as psA,
            tc.tile_pool(name="psB", bufs=2, space="PSUM") as psB,
        ):
            # Startup-critical loads. DMA ring throughput scales with the
            # per-partition line size (2KB lines ~80GB/s, 4KB ~150GB/s, 8KB
            # ~200+GB/s), so keep transfers WHOLE: x (8KB lines) rides the
            # sync ring (up ~8.7us), w13[0] (4KB lines) the scalar ring
            # (~9.9us, after its ACT table load). Both land ~14us.
            # high_priority pins these triggers at the head of their engine
            # queues — in particular ahead of the two ACT_TABLE_LOADs
            # (2.56us) the framework schedules on the scalar queue, which
            # otherwise delay the w13[0] descriptor submission.
            with tc.high_priority():
                xsb = xp.tile([P, KH, C], BF16)
                nc.sync.dma_start(xsb[:], xg[:])
                # ii=0 alone (512KB): keeps the scalar-ring transfer short
                # so even with ring-start jitter the warmup->data gap stays
                # under the ~3.4us HAM idle window (a 1MB pair here was
                # observed to blow past it and re-throttle the PE clock).
                w13sb0 = wp.tile([P, 2, KH, P], BF16, tag="w13", bufs=4)
                nc.scalar.dma_start(w13sb0[:], w13t[0][:, 0])

            def xh(kh):
                return xsb[:, kh, :]

            # PE warmup: ramp the tensor engine to high-activity clock while
            # the input DMAs are in flight. Reads a zeroed tile, result is
            # never consumed.
            wtile = warm.tile([P, W_COLS], BF16)
            nc.vector.memset(wtile[:], 0.0)
            # Shares the Phase B psum pool (tag "py"): warmup is long done
            # before Phase B allocates its first chain psum.
            wps = psB.tile([P, W_COLS], F32, tag="py")
            for i in range(N_WARM):
                nc.tensor.matmul(
                    wps, wtile[:, :P], wtile[:], start=(i == 0),
                    stop=(i == N_WARM - 1),
                )

            psb = pp.tile([P, II, C], BF16)

            # Phase A: h1 = silu(x@w1), h3 = x@w3, p = h1*h3 (all transposed)
            # w13 stream: ii=1..3 as single tiles on the gpsimd ring (just-in-
            # time at startup), ii>=4 as pair tiles (one DMA per two ii —
            # fewer triggers and semaphores). The scalar ring, free once the
            # startup x chunks land, carries most pairs; gpsimd the rest.
            wpair = None
            for ii in range(II):
                j, a = divmod(ii, 2)
                if ii == 0:
                    wsel = lambda half, kh: w13sb0[:, half, kh, :]
                elif ii in (1, 2, 3):
                    # Just-in-time singles: ii=1,2 on the gpsimd software-DGE
                    # ring (its first data lands ~15us, one tile per ~3.4us);
                    # ii=3 rides the scalar ring right behind w13[0] — the
                    # gpsimd ring delivers it ~1us too late.
                    wsb = wp.tile([P, 2, KH, P], BF16, tag="w13", bufs=4)
                    eng = nc.scalar if ii == 3 else nc.gpsimd
                    eng.dma_start(wsb[:], w13t[j][:, a])
                    wsel = lambda half, kh, t=wsb: t[:, half, kh, :]
                elif a == 0:
                    wpair = wp.tile([P, 2, 2, KH, P], BF16, tag="w13p", bufs=5)
                    # j=2,3 on scalar (gpsimd is busy with the JIT singles
                    # until ~22us); afterwards alternate rings.
                    eng = nc.scalar if (j < 4 or j % 2 == 1) else nc.gpsimd
                    eng.dma_start(wpair[:], w13t[j])
                    wsel = lambda half, kh, t=wpair: t[:, 0, half, kh, :]
                else:
                    wsel = lambda half, kh, t=wpair: t[:, 1, half, kh, :]
                pg = psA.tile([P, C], F32, tag="pg")
                pu = psA.tile([P, C], F32, tag="pu")
                for kh in range(KH):
                    nc.tensor.matmul(
                        pg,
                        wsel(0, kh),
                        xh(kh),
                        start=(kh == 0),
                        stop=(kh == KH - 1),
                    )
                for kh in range(KH):
                    nc.tensor.matmul(
                        pu,
                        wsel(1, kh),
                        xh(kh),
                        start=(kh == 0),
                        stop=(kh == KH - 1),
                    )
                gs = gp.tile([P, C], BF16, tag="g")
                nc.scalar.activation(gs, pg, mybir.ActivationFunctionType.Silu)
                nc.vector.tensor_tensor(
                    psb[:, ii, :], gs, pu, mybir.AluOpType.mult
                )

            # Phase B: y = p @ w2 (transposed: yT = w2T-contraction over I).
            # w2 arrives as 1MB pair tiles (hh 2q, 2q+1 together) and y goes
            # out as pair stores — fewer DMA triggers and semaphores (the
            # teardown zeroes every semaphore inside the measured window).
            # The last hh is split column-wise so its first half's copy+DMA
            # overlaps the second half's matmuls (shorter kernel tail).
            w2sb = yd = None
            for hh in range(KH):
                q, b = divmod(hh, 2)
                if b == 0:
                    w2sb = w2p.tile([P, 2, II, P], BF16, tag="w2")
                    # first pair on scalar (its w13 stream drains first at
                    # the A->B transition; gpsimd is still backlogged),
                    # then alternate.
                    eng = nc.scalar if q % 2 == 0 else nc.gpsimd
                    eng.dma_start(w2sb[:], w2t[q])
                    yd = yp.tile([P, 2, C], F32, tag="y2")
                halves = [(0, C)] if hh < KH - 1 else [
                    (0, C // 2), (C // 2, C - C // 2),
                ]
                for hi, (c0, cc) in enumerate(halves):
                    py = psB.tile([P, cc], F32, tag="py")
                    for ik in range(II):
                        nc.tensor.matmul(
                            py,
                            w2sb[:, b, ik, :],
                            psb[:, ik, c0 : c0 + cc],
                            start=(ik == 0),
                            stop=(ik == II - 1),
                        )
                    # DVE copies keep the COPY activation table off the
                    # scalar queue (its ACT_TABLE_LOAD would delay the scalar
                    # DMA ring's startup-critical triggers by ~1.3us).
                    if hh < KH - 1 or hi == 0:
                        nc.vector.tensor_copy(yd[:, b, c0 : c0 + cc], py)
                        if b == 1 and hh < KH - 1:
                            nc.sync.dma_start(yt[q], yd[:])
                        elif hh == KH - 1:
                            # penultimate store: hh=6 whole + hh=7 first
                            # half, one trigger
                            nc.sync.dma_start(
                                yt[q, :, 0, :], yd[:, 0, :]
                            )
                            nc.sync.dma_start(
                                yt[q, :, 1, c0 : c0 + cc],
                                yd[:, 1, c0 : c0 + cc],
                            )
                    else:
                        # Final piece on the other hardware DGE ring,
                        # pipelined behind the first half's store.
                        yb = yp.tile([P, cc], F32, tag="y")
                        nc.vector.tensor_copy(yb, py)
                        nc.scalar.dma_start(
                            yt[q, :, 1, c0 : c0 + cc], yb[:]
                        )

    nc.compile()
    return nc


_PROGRAM_CACHE = {}


def _host_swiglu(x, w1e, w2e, w3e):
    g = x @ w1e
    u = x @ w3e
    g = g / (1.0 + np.exp(-g))
    return (g * u) @ w2e


def kernel(x, expert_indices, expert_weights, w1, w2, w3):
    x = np.asarray(x, dtype=np.float32)
    idx = np.asarray(expert_indices)
    wts = np.asarray(expert_weights, dtype=np.float32)
    w1 = np.asarray(w1, dtype=np.float32)
    w2 = np.asarray(w2, dtype=np.float32)
    w3 = np.asarray(w3, dtype=np.float32)
    N = x.shape[0]
    E = w1.shape[0]
    bf16 = ml_dtypes.bfloat16

    # host-side routing: token list (with multiplicity) per expert; tokens
    # beyond CAP spill to the host f32 path (tiny tail, keeps device at one
    # full-width PE chunk)
    toks, tokw, spill_toks, spill_w = [], [], [], []
    for e in range(E):
        rows, cols = np.nonzero(idx == e)
        w_e = wts[rows, cols]
        toks.append(rows[:CAP])
        tokw.append(w_e[:CAP])
        spill_toks.append(rows[CAP:])
        spill_w.append(w_e[CAP:])
    C = max(16, max(len(t) for t in toks))
    C = ((C + 15) // 16) * 16

    if C not in _PROGRAM_CACHE:
        _PROGRAM_CACHE[C] = _build(C)
    nc = _PROGRAM_CACHE[C]

    in_maps = []
    for e in range(E):
        xt = np.zeros((C, H), dtype=np.float32)
        if len(toks[e]):
            xt[: len(toks[e])] = x[toks[e]]
        # [C, H] -> [hp, kh, c]
        xge = xt.T.reshape(KH, P, C).transpose(1, 0, 2)
        # w1/w3 [H, I] -> [ii, hp, {w1,w3}, kh, m] -> pair-major
        # [ii/2, hp, ii%2, {w1,w3}, kh, m]
        w13 = np.stack(
            [
                w1[e].reshape(KH, P, II, P).transpose(2, 1, 0, 3),
                w3[e].reshape(KH, P, II, P).transpose(2, 1, 0, 3),
            ],
            axis=2,
        )  # [II, P, 2, KH, P]
        w13 = w13.reshape(II // 2, 2, P, 2, KH, P).swapaxes(1, 2)
        in_maps.append(
            {
                "xg": np.ascontiguousarray(xge.astype(bf16)),
                "w13t": np.ascontiguousarray(w13.astype(bf16)),
                "w2t": np.ascontiguousarray(
                    w2[e].reshape(II, P, KH, P).transpose(2, 1, 0, 3)
                    .reshape(KH // 2, 2, P, II, P).swapaxes(1, 2).astype(bf16)
                ),
            }
        )

    res = run_bass_kernel_spmd(nc, in_maps, core_ids=list(range(E)))

    out = np.zeros((N, H), dtype=np.float32)
    for e in range(E):
        cnt = len(toks[e])
        if cnt:
            y = (
                res.results[e]["yt"]
                .reshape(KH // 2, P, 2, C)
                .swapaxes(1, 2)
                .reshape(H, C)
                .T[:cnt]
            )
            np.add.at(out, toks[e], y * tokw[e][:, None])
        if len(spill_toks[e]):
            ys = _host_swiglu(x[spill_toks[e]], w1[e], w2[e], w3[e])
            np.add.at(out, spill_toks[e], ys * spill_w[e][:, None])
    return out
